# revision 2
# baseline (speedup 1.0000x reference)
"""Max-plus layer v2: LSE-via-matmul with DVE bit-trick exp/log (no ACT engine,
no table loads), f16 I/O, PE pre-warming, single-launch-latency optimized.

y[b,i] = max_j(x[b,j] + a[i,j]) + bias[i]
       ~= mx[b] + (1/t)*ln( sum_j exp(t*(x[b,j]-mx[b])) * exp(t*(a[i,j]+bias[i])) )

Device chain per core (128 batch rows):
  DVE  nmx = -rowmax(x16)                                  (reduce, f16 2x rate)
  DVE  nmx2 = nmx + C1 ; nmx3 = nmx + C3                   (tiny [128,1] ops)
  DVE  u16 = sat_u16((x + nmx2) * s)  -> bf16 bit pattern  (fast exp2; underflow -> +0)
  PE   transpose u (4 blocks, bf16) -> PSUM
  DVE  copy u^T PSUM -> SBUF
  PE   S = u^T.T @ v^T (4 matmuls K=128, N=512, f32 PSUM)
  DVE  y16 = bits(S)*c2 - nmx3      (fast log2 + affine fused; f16 out)

PE is kept busy with dummy transposes from ~0.8us so the real matmuls run at
the warm 2.4 GHz clock. The identity matrix is generated on-chip (Pool
iota/affine_select). No ScalarE activations anywhere -> no ACT table loads.
"""

import sys

sys.path.insert(0, "/opt/trn_rl_repo")

import ml_dtypes
import numpy as np

import concourse.mybir as mybir
import concourse.tile as tile
from concourse import bacc
from concourse.bass_utils import run_bass_kernel_spmd

F32 = mybir.dt.float32
BF16 = mybir.dt.bfloat16
F16 = mybir.dt.float16
U16 = mybir.dt.uint16
I32 = mybir.dt.int32

B = 1024
J = 512
O = 512
N_CORES = 8
B_SH = B // N_CORES  # 128
NQ = J // 128  # 4

T = 192.0
LOG2E = 1.4426950408889634
LN2 = 0.6931471805599453
SIG_E = 0.043  # fast-exp2 mantissa offset
SIG_L = 0.043  # fast-log2 mantissa offset
EB = 0.0025  # global bias to center the error distribution

S_SCALE = T * LOG2E * 128.0  # f32 -> bf16-bit units
C1 = (127.0 - SIG_E) * 128.0 / S_SCALE
C2 = LN2 / (T * 8388608.0)  # ln2 / (t * 2^23)  (f32-bits log path)
C2B = LN2 / (T * 128.0)  # ln2 / (t * 2^7)   (bf16-bits log path)
C3 = (127.0 - SIG_L) * LN2 / T + EB

TRACE = False
LAST_RESULTS = None
_nc_cache = None


def _build_bass(reps: int = 1, loop_reps: int = 1, warmups: int = 30, vt_split: int = 2):
    nc = bacc.Bacc("TRN2", target_bir_lowering=False, debug=False, num_devices=N_CORES)
    x_t = nc.dram_tensor("x", [B_SH, J], F16, kind="ExternalInput")
    vt_t = nc.dram_tensor("vt", [128, NQ, O], BF16, kind="ExternalInput")
    y_t = nc.dram_tensor("y", [B_SH, O], F16, kind="ExternalOutput")

    with tile.TileContext(nc) as tc:
        with (
            tc.tile_pool(name="sb", bufs=1) as sb,
            tc.tile_pool(name="ps", bufs=1, space="PSUM") as ps,
        ):
            x_sb = sb.tile([128, J], F16)
            vt_sb = sb.tile([128, NQ, O], BF16)
            ones = sb.tile([128, 128], BF16)
            ident = sb.tile([128, 128], BF16)
            nmx = sb.tile([128, 1], F32)
            nmx2 = sb.tile([128, 1], F32)
            nmx3 = sb.tile([128, 1], F32)
            u_sb = sb.tile([128, J], BF16)
            ut_sb = sb.tile([128, NQ, 128], BF16)
            y_sb = sb.tile([128, O], F16)
            ps_w = ps.tile([128, 128], BF16)  # warmup scratch
            ps_t = [ps.tile([128, 2, 128], BF16, name=f"ps_t{h}") for h in range(2)]
            ps_y = [ps.tile([128, O // 2], F32, name=f"ps_y{h}") for h in range(2)]

            # input DMAs (SP ring, in priority order)
            nc.sync.dma_start(x_sb[:], x_t.ap())
            if vt_split <= 1:
                nc.sync.dma_start(vt_sb[:], vt_t.ap())
            else:
                step = NQ // vt_split
                for h in range(vt_split):
                    nc.sync.dma_start(
                        vt_sb[:, h * step : (h + 1) * step, :],
                        vt_t.ap()[:, h * step : (h + 1) * step, :],
                    )

            # on-chip identity (Pool): ones, then keep diag via affine iota p-f==0
            nc.gpsimd.memset(ones[:], 1.0)
            nc.gpsimd.affine_select(
                ident[:],
                ones[:],
                pattern=[[-1, 128]],
                compare_op=mybir.AluOpType.is_equal,
                fill=0.0,
                base=0,
                channel_multiplier=1,
            )

            # PE warm-up: keep the clock un-gated until the real matmuls
            for _ in range(warmups):
                nc.tensor.transpose(ps_w[:], ones[:], ident[:])

            def body():
                nc.vector.tensor_reduce(
                    nmx[:], x_sb[:], mybir.AxisListType.X, mybir.AluOpType.max,
                    negate=True,
                )
                nc.vector.tensor_scalar(
                    out=nmx2[:], in0=nmx[:], scalar1=float(C1), scalar2=None,
                    op0=mybir.AluOpType.add,
                )
                # u16 = sat_u16((x + nmx2) * s)  (bf16 bits; negatives clamp to +0)
                nc.vector.tensor_scalar(
                    out=u_sb[:].bitcast(U16), in0=x_sb[:], scalar1=nmx2[:],
                    scalar2=float(S_SCALE), op0=mybir.AluOpType.add,
                    op1=mybir.AluOpType.mult,
                )
                nc.vector.tensor_scalar(
                    out=nmx3[:], in0=nmx[:], scalar1=float(C3), scalar2=None,
                    op0=mybir.AluOpType.add,
                )
                # transpose all 4 blocks -> one PSUM->SBUF copy
                for q in range(NQ):
                    nc.tensor.transpose(
                        ps_t[q // 2][:, q % 2, :], u_sb[:, q * 128 : (q + 1) * 128],
                        ident[:],
                    )
                nc.vector.tensor_copy(out=ut_sb[:, 0:2, :], in_=ps_t[0][:])
                nc.vector.tensor_copy(out=ut_sb[:, 2:4, :], in_=ps_t[1][:])
                for h in range(2):
                    for q in range(NQ):
                        nc.tensor.matmul(
                            ps_y[h][:],
                            lhsT=ut_sb[:, q, :],
                            rhs=vt_sb[:, q, h * 256 : (h + 1) * 256],
                            start=(q == 0),
                            stop=(q == NQ - 1),
                        )
                    # y = bits(S)*c2 - nmx3  (fast log2 + affine + f16 cast)
                    nc.vector.tensor_scalar(
                        out=y_sb[:, h * 256 : (h + 1) * 256],
                        in0=ps_y[h][:].bitcast(I32), scalar1=float(C2),
                        scalar2=nmx3[:], op0=mybir.AluOpType.mult,
                        op1=mybir.AluOpType.subtract,
                    )

            if loop_reps > 1:
                with tc.For_i(0, loop_reps, 1):
                    body()
            else:
                body()

            nc.sync.dma_start(y_t.ap(), y_sb[:])
    nc.compile()
    return nc


def _prep_inputs(x, a, bias):
    """Host prep: fold bias, exponentiate weights to bf16, transpose."""
    a_p = a.astype(np.float64) + bias.astype(np.float64)[:, None]
    v = np.exp(T * a_p).astype(ml_dtypes.bfloat16)  # [O, J]
    vt = np.ascontiguousarray(v.T.reshape(NQ, 128, O).transpose(1, 0, 2))
    x16 = x.astype(np.float16)

    in_maps = []
    for c in range(N_CORES):
        in_maps.append(
            {
                "x": np.ascontiguousarray(x16[c * B_SH : (c + 1) * B_SH]),
                "vt": vt,
            }
        )
    return in_maps


def kernel(x, a, bias):
    global _nc_cache, LAST_RESULTS
    x = np.ascontiguousarray(np.asarray(x, dtype=np.float32))
    a = np.asarray(a, dtype=np.float32)
    bias = np.asarray(bias, dtype=np.float32)
    assert x.shape == (B, J) and a.shape == (O, J) and bias.shape == (O,)

    if _nc_cache is None:
        _nc_cache = _build_bass()
    nc = _nc_cache

    in_maps = _prep_inputs(x, a, bias)
    res = run_bass_kernel_spmd(nc, in_maps, core_ids=list(range(N_CORES)), trace=TRACE)
    LAST_RESULTS = res
    y = np.concatenate(
        [res.results[c]["y"].astype(np.float32) for c in range(N_CORES)], axis=0
    )
    return y


# revision 3
# speedup vs baseline: 1.0140x; 1.0140x over previous
"""Max-plus layer v2: LSE-via-matmul with DVE bit-trick exp/log (no ACT engine,
no table loads), f16 I/O, PE pre-warming, single-launch-latency optimized.

y[b,i] = max_j(x[b,j] + a[i,j]) + bias[i]
       ~= mx[b] + (1/t)*ln( sum_j exp(t*(x[b,j]-mx[b])) * exp(t*(a[i,j]+bias[i])) )

Device chain per core (128 batch rows):
  DVE  nmx = -rowmax(x16)                                  (reduce, f16 2x rate)
  DVE  nmx2 = nmx + C1 ; nmx3 = nmx + C3                   (tiny [128,1] ops)
  DVE  u16 = sat_u16((x + nmx2) * s)  -> bf16 bit pattern  (fast exp2; underflow -> +0)
  PE   transpose u (4 blocks, bf16) -> PSUM
  DVE  copy u^T PSUM -> SBUF
  PE   S = u^T.T @ v^T (4 matmuls K=128, N=512, f32 PSUM)
  DVE  y16 = bits(S)*c2 - nmx3      (fast log2 + affine fused; f16 out)

PE is kept busy with dummy transposes from ~0.8us so the real matmuls run at
the warm 2.4 GHz clock. The identity matrix is generated on-chip (Pool
iota/affine_select). No ScalarE activations anywhere -> no ACT table loads.
"""

import sys

sys.path.insert(0, "/opt/trn_rl_repo")

import ml_dtypes
import numpy as np

import concourse.mybir as mybir
import concourse.tile as tile
from concourse import bacc
from concourse.bass_utils import run_bass_kernel_spmd

F32 = mybir.dt.float32
BF16 = mybir.dt.bfloat16
F16 = mybir.dt.float16
U16 = mybir.dt.uint16
I32 = mybir.dt.int32

B = 1024
J = 512
O = 512
N_CORES = 8
B_SH = B // N_CORES  # 128
NQ = J // 128  # 4

T = 192.0
LOG2E = 1.4426950408889634
LN2 = 0.6931471805599453
SIG_E = 0.043  # fast-exp2 mantissa offset
SIG_L = 0.043  # fast-log2 mantissa offset
EB = 0.0025  # global bias to center the error distribution

S_SCALE = T * LOG2E * 128.0  # f32 -> bf16-bit units
C1 = (127.0 - SIG_E) * 128.0 / S_SCALE
C2 = LN2 / (T * 8388608.0)  # ln2 / (t * 2^23)  (f32-bits log path)
C2B = LN2 / (T * 128.0)  # ln2 / (t * 2^7)   (bf16-bits log path)
C3 = (127.0 - SIG_L) * LN2 / T + EB

TRACE = False
LAST_RESULTS = None
_nc_cache = None


def _build_bass(
    reps: int = 1,
    loop_reps: int = 1,
    warmups: int = 22,
    vt_split: int = 2,
    skip_preamble: bool = True,
):
    if skip_preamble:
        # Bass.__init__ emits 4 const-tile memsets + an all-engine barrier
        # (~0.7us) that nothing in this kernel consumes; suppress them for
        # construction only. Engine streams still start in lockstep at NEFF
        # exec; all cross-engine ordering here is via Tile-assigned sems.
        import concourse.bass as bass_mod

        orig_barrier = bass_mod.Bass.all_engine_barrier
        orig_memset = bass_mod.BassGpSimd.memset
        bass_mod.Bass.all_engine_barrier = lambda self, **kw: None
        bass_mod.BassGpSimd.memset = lambda self, ap, c: None
        try:
            nc = bacc.Bacc(
                "TRN2", target_bir_lowering=False, debug=False, num_devices=N_CORES
            )
        finally:
            bass_mod.Bass.all_engine_barrier = orig_barrier
            bass_mod.BassGpSimd.memset = orig_memset
    else:
        nc = bacc.Bacc(
            "TRN2", target_bir_lowering=False, debug=False, num_devices=N_CORES
        )
    x_t = nc.dram_tensor("x", [B_SH, J], F16, kind="ExternalInput")
    vt_t = nc.dram_tensor("vt", [128, NQ, O], BF16, kind="ExternalInput")
    y_t = nc.dram_tensor("y", [B_SH, O], F16, kind="ExternalOutput")

    with tile.TileContext(nc) as tc:
        with (
            tc.tile_pool(name="sb", bufs=1) as sb,
            tc.tile_pool(name="ps", bufs=1, space="PSUM") as ps,
        ):
            x_sb = sb.tile([128, J], F16)
            vt_sb = sb.tile([128, NQ, O], BF16)
            ones = sb.tile([128, 128], BF16)
            ident = sb.tile([128, 128], BF16)
            nmx = sb.tile([128, 1], F32)
            nmx2 = sb.tile([128, 1], F32)
            nmx3 = sb.tile([128, 1], F32)
            u_sb = sb.tile([128, J], BF16)
            ut_sb = sb.tile([128, NQ, 128], BF16)
            y_sb = sb.tile([128, O], F16)
            ps_w = ps.tile([128, 128], BF16)  # warmup scratch
            ps_t = [ps.tile([128, 2, 128], BF16, name=f"ps_t{h}") for h in range(2)]
            ps_y = [ps.tile([128, O // 2], F32, name=f"ps_y{h}") for h in range(2)]

            # input DMAs (SP ring, in priority order)
            nc.sync.dma_start(x_sb[:], x_t.ap())
            if vt_split <= 1:
                nc.sync.dma_start(vt_sb[:], vt_t.ap())
            else:
                step = NQ // vt_split
                for h in range(vt_split):
                    nc.sync.dma_start(
                        vt_sb[:, h * step : (h + 1) * step, :],
                        vt_t.ap()[:, h * step : (h + 1) * step, :],
                    )

            # on-chip identity (Pool): ones, then keep diag via affine iota p-f==0
            nc.gpsimd.memset(ones[:], 1.0)
            nc.gpsimd.affine_select(
                ident[:],
                ones[:],
                pattern=[[-1, 128]],
                compare_op=mybir.AluOpType.is_equal,
                fill=0.0,
                base=0,
                channel_multiplier=1,
            )

            # PE warm-up: keep the clock un-gated until the real matmuls
            for _ in range(warmups):
                nc.tensor.transpose(ps_w[:], ones[:], ident[:])

            def body():
                nc.vector.tensor_reduce(
                    nmx[:], x_sb[:], mybir.AxisListType.X, mybir.AluOpType.max,
                    negate=True,
                )
                nc.vector.tensor_scalar(
                    out=nmx2[:], in0=nmx[:], scalar1=float(C1), scalar2=None,
                    op0=mybir.AluOpType.add,
                )
                # u16 = sat_u16((x + nmx2) * s)  (bf16 bits; negatives clamp to +0)
                nc.vector.tensor_scalar(
                    out=u_sb[:].bitcast(U16), in0=x_sb[:], scalar1=nmx2[:],
                    scalar2=float(S_SCALE), op0=mybir.AluOpType.add,
                    op1=mybir.AluOpType.mult,
                )
                nc.vector.tensor_scalar(
                    out=nmx3[:], in0=nmx[:], scalar1=float(C3), scalar2=None,
                    op0=mybir.AluOpType.add,
                )
                # transpose all 4 blocks -> one PSUM->SBUF copy
                for q in range(NQ):
                    nc.tensor.transpose(
                        ps_t[q // 2][:, q % 2, :], u_sb[:, q * 128 : (q + 1) * 128],
                        ident[:],
                    )
                nc.vector.tensor_copy(out=ut_sb[:, 0:2, :], in_=ps_t[0][:])
                nc.vector.tensor_copy(out=ut_sb[:, 2:4, :], in_=ps_t[1][:])
                for h in range(2):
                    for q in range(NQ):
                        nc.tensor.matmul(
                            ps_y[h][:],
                            lhsT=ut_sb[:, q, :],
                            rhs=vt_sb[:, q, h * 256 : (h + 1) * 256],
                            start=(q == 0),
                            stop=(q == NQ - 1),
                        )
                    # y = bits(S)*c2 - nmx3  (fast log2 + affine + f16 cast)
                    nc.vector.tensor_scalar(
                        out=y_sb[:, h * 256 : (h + 1) * 256],
                        in0=ps_y[h][:].bitcast(I32), scalar1=float(C2),
                        scalar2=nmx3[:], op0=mybir.AluOpType.mult,
                        op1=mybir.AluOpType.subtract,
                    )

            if loop_reps > 1:
                with tc.For_i(0, loop_reps, 1):
                    body()
            else:
                body()

            nc.sync.dma_start(y_t.ap(), y_sb[:])
    nc.compile()
    return nc


def _prep_inputs(x, a, bias):
    """Host prep: fold bias, exponentiate weights to bf16, transpose."""
    a_p = a.astype(np.float64) + bias.astype(np.float64)[:, None]
    v = np.exp(T * a_p).astype(ml_dtypes.bfloat16)  # [O, J]
    vt = np.ascontiguousarray(v.T.reshape(NQ, 128, O).transpose(1, 0, 2))
    x16 = x.astype(np.float16)

    in_maps = []
    for c in range(N_CORES):
        in_maps.append(
            {
                "x": np.ascontiguousarray(x16[c * B_SH : (c + 1) * B_SH]),
                "vt": vt,
            }
        )
    return in_maps


def kernel(x, a, bias):
    global _nc_cache, LAST_RESULTS
    x = np.ascontiguousarray(np.asarray(x, dtype=np.float32))
    a = np.asarray(a, dtype=np.float32)
    bias = np.asarray(bias, dtype=np.float32)
    assert x.shape == (B, J) and a.shape == (O, J) and bias.shape == (O,)

    if _nc_cache is None:
        _nc_cache = _build_bass()
    nc = _nc_cache

    in_maps = _prep_inputs(x, a, bias)
    res = run_bass_kernel_spmd(nc, in_maps, core_ids=list(range(N_CORES)), trace=TRACE)
    LAST_RESULTS = res
    y = np.concatenate(
        [res.results[c]["y"].astype(np.float32) for c in range(N_CORES)], axis=0
    )
    return y
